# revision 9
# baseline (speedup 1.0000x reference)
"""PSENet-style OHEM + dice loss on 8 Trainium2 NeuronCores.

Data-parallel over the batch: core b processes image b entirely on-chip.
HBM traffic is minimized by staging inputs at low precision (tolerance is
rel-err < 2e-2): logits x as fp8 e3m4, labels g as fp8 e4m3 (0/1 exact),
training mask m as fp16 -- 6.6 MB/image instead of 24.6 MB fp32.

Per-channel masked dice sums (labels g, masks m/M are exactly 0/1):
  sig_k := sigmoid(x_k + (M-1)*BIG) = sigmoid(x_k)*M  (up to sig(-44) ~ 1e-19)
  a_k = sum(sig_k * g_k),  b_k = sum(sig_k^2),  c_k = sum(g_k * M)
The (M-1)*BIG mask-offset is applied *during the DMA*: the destination tile
is prefilled with (M-1)*BIG by a 4x-rate DVE tensor_scalar, and the x DMA
uses accum_op=add with an fp8->fp16 cast (SWDGE CCE). The text-channel mask
M = (x6>0)*m falls out as (xm6>0) of the already-offset text logits.

Dot-product reductions run mostly on the otherwise-idle TensorEngine via
chunked [128,128] accumulating matmuls whose PSUM diagonal holds the
answer (extracted by one DVE I-mask STT+accum per reduction):
  - all six c_k share M as the stationary operand and stream 3 g channels
    per matmul ([128,384] PSUM tiles, g consumed directly as fp8)
  - b_k/a_k share sig_k as the stationary operand
DVE reduce-class ops (accum_out) run at 1x regardless of dtype, so only 3
a_k reductions live on DVE; ACT does sigmoids + one square-accum.

Engine occupancy (cost-model, per image): PE ~23us, DVE ~22us, ACT ~23us,
Pool (SWDGE issue) ~17us, DMA queues ~17us.  Final scalars: accumulator
columns reduced across partitions by one ones-matmul; host combines
8 x 26 floats (OHEM fast path verified on host: sel == training_mask iff
(RATIO+1)*pos_num >= N, with ~28 sigma of margin; exact host fallback
otherwise).
"""

import os
import sys

import numpy as np
import ml_dtypes

for _p in ("/opt/trn_rl_repo", "/root/.axon_site/_ro/trn_rl_repo"):
    if os.path.isdir(_p) and _p not in sys.path:
        sys.path.append(_p)

import concourse.bacc as bacc
import concourse.tile as tile
from concourse import mybir
from concourse.bass_utils import run_bass_kernel_spmd

B, C, H, W = 8, 7, 640, 640
NK = C - 1            # kernel channels
N = H * W             # pixels per image
P = 128               # SBUF partitions
F = N // P            # free dim per plane tile (3200)
NCH = F // P          # 25 matmul chunks per plane reduction
BIG = 50.0
NCORES = 8
LAMBDA = 0.7
RATIO = 3

_f32 = mybir.dt.float32
_f16 = mybir.dt.float16
_f8x = mybir.dt.float8e3   # logits (e3m4: 4-bit mantissa, |x| < 15.5)
_f8g = mybir.dt.float8e4   # labels (0/1 exact)
_AF = mybir.ActivationFunctionType
_ALU = mybir.AluOpType

# accumulator column map (acc_dve [128, 24]; b6 -> acc_act col 0)
A_COL = {k: k for k in range(7)}
B_COL = {0: 7, 1: 8, 2: 9, 3: 10, 4: 11, 5: 12}
C_COL = {k: 14 + k for k in range(7)}
A_DVE = (0, 1, 6)          # a_k on DVE STT+accum; a2..a5 paired with b on PE
X_SLABS = [[0], [1, 2], [3, 4, 5]]   # kernel-channel x DMA grouping


def _plane(dram_ap):
    """[H, W] dram slab -> [128, 3200] partition-major access pattern."""
    return dram_ap.rearrange("(p q) w -> p (q w)", p=P)


def _slab(dram_ap):
    """[n, H, W] dram slab -> [128, n, 3200] (channel-major free dims)."""
    return dram_ap.rearrange("c (p q) w -> p c (q w)", p=P)


def build_nc(debug=False, reps=1):
    nc = bacc.Bacc("TRN2", target_bir_lowering=False, debug=debug)
    x_d = nc.dram_tensor("x", [C, H, W], _f8x, kind="ExternalInput")
    g_d = nc.dram_tensor("g", [C, H, W], _f8g, kind="ExternalInput")
    m_d = nc.dram_tensor("m", [H, W], _f16, kind="ExternalInput")
    res_d = nc.dram_tensor("res", [26, 1], _f32, kind="ExternalOutput")

    with (
        tile.TileContext(nc) as tc,
        tc.tile_pool(name="const", bufs=1) as cpool,
        tc.tile_pool(name="xin", bufs=2) as xpool,
        tc.tile_pool(name="gin", bufs=2) as gpool,
        tc.tile_pool(name="sigp", bufs=3) as spool,
        tc.tile_pool(name="mp", bufs=2) as mpool,
        tc.tile_pool(name="junk", bufs=2) as jpool,
        tc.tile_pool(name="j128", bufs=2) as j128pool,
        tc.tile_pool(name="psba", bufs=2, space="PSUM") as bapool,
        tc.tile_pool(name="psc", bufs=1, space="PSUM") as cpool_ps,
        tc.tile_pool(name="psf", bufs=1, space="PSUM") as pfpool,
    ):
        ones32 = cpool.tile([P, 1], _f32)
        nc.gpsimd.memset(ones32[:], 1.0)
        iota_t = cpool.tile([P, P], mybir.dt.int32)
        nc.gpsimd.iota(iota_t[:], pattern=[[1, P]], base=0, channel_multiplier=-1)
        I_t = cpool.tile([P, P], _f16)
        nc.vector.tensor_scalar(I_t[:], iota_t[:], 0.0, None, _ALU.is_equal)
        acc_dve = cpool.tile([P, 24], _f32)
        acc_act = cpool.tile([P, 2], _f32)
        nc.vector.memset(acc_dve[:], 0.0)
        nc.scalar.memzero(acc_act[:])

        def image_body(rep):
            def extract(ps_slice, col, tag):
                jk = j128pool.tile([P, P], _f16, tag="j128", name=f"x{tag}_r{rep}")
                nc.vector.scalar_tensor_tensor(
                    jk[:], ps_slice, 1.0, I_t[:], _ALU.mult, _ALU.mult,
                    accum_out=acc_dve[:, col:col + 1],
                )

            def pe_reduce(u, v, col, tag):
                ps = bapool.tile([P, P], _f32, tag="bps", name=f"{tag}_r{rep}")
                for t in range(NCH):
                    nc.tensor.matmul(
                        ps[:], lhsT=u[:, t * P:(t + 1) * P],
                        rhs=v[:, t * P:(t + 1) * P],
                        start=(t == 0), stop=(t == NCH - 1),
                    )
                extract(ps[:], col, tag)

            def pe_reduce_ba(sig, gsl, k, j):
                # b_k = <sig,sig>, a_k = <sig,g>; sig is the shared stationary
                bps = bapool.tile([P, P], _f32, tag="bps", name=f"b{k}_r{rep}")
                aps = bapool.tile([P, P], _f32, tag="aps", name=f"a{k}_r{rep}")
                for t in range(NCH):
                    sl = sig[:, t * P:(t + 1) * P]
                    nc.tensor.matmul(bps[:], lhsT=sl, rhs=sl,
                                     start=(t == 0), stop=(t == NCH - 1))
                    nc.tensor.matmul(
                        aps[:], lhsT=sl,
                        rhs=gsl[:, j * F + t * P:j * F + (t + 1) * P],
                        start=(t == 0), stop=(t == NCH - 1),
                    )
                extract(bps[:], B_COL[k], f"b{k}")
                extract(aps[:], A_COL[k], f"a{k}")

            # ---- text channel ----
            def add_dma(dst, k):
                # CCE accumulate corrupts beyond 2048 elems/partition-row:
                # split each plane into two 1600-column halves.
                src = _plane(x_d.ap()[k])
                Fh = F // 2
                for h in range(2):
                    nc.gpsimd.dma_start(
                        dst[:, h * Fh:(h + 1) * Fh],
                        src[:, h * Fh:(h + 1) * Fh],
                        accum_op=_ALU.add,
                    )

            m_t = mpool.tile([P, F], _f16, tag="m", name=f"m_r{rep}")
            nc.sync.dma_start(m_t[:], _plane(m_d.ap()))
            xm6 = xpool.tile([P, F], _f16, tag="x6", name=f"x6_r{rep}")
            nc.vector.tensor_scalar(
                xm6[:], m_t[:], BIG, -BIG, _ALU.mult, _ALU.add
            )
            add_dma(xm6, C - 1)
            g6 = gpool.tile([P, F], _f8g, tag="g6", name=f"g6_r{rep}")
            nc.sync.dma_start(g6[:], _plane(g_d.ap()[C - 1]))

            sig6 = spool.tile([P, F], _f16, tag="sig", name=f"sig6_r{rep}")
            nc.scalar.activation(sig6[:], xm6[:], _AF.Sigmoid)
            M_t = mpool.tile([P, F], _f16, tag="M", name=f"M_r{rep}")
            nc.vector.tensor_scalar(M_t[:], xm6[:], 0.0, None, _ALU.is_gt)

            # b6 on ACT; a6 on DVE; c6 = <g6, m> on PE
            jb6 = jpool.tile([P, F], _f16, tag="junk", name=f"jb6_r{rep}")
            nc.scalar.activation(jb6[:], sig6[:], _AF.Square,
                                 accum_out=acc_act[:, 0:1])
            ja6 = jpool.tile([P, F], _f16, tag="junk", name=f"ja6_r{rep}")
            nc.vector.scalar_tensor_tensor(
                ja6[:], g6[:], 1.0, sig6[:], _ALU.mult, _ALU.mult,
                accum_out=acc_dve[:, A_COL[6]:A_COL[6] + 1],
            )
            pe_reduce(g6, m_t, C_COL[6], "c6")

            # ---- kernel-channel g slabs (raw fp8, HWDGE) ----
            gs = []
            for si in range(2):
                gt = gpool.tile([P, 3 * F], _f8g, tag=f"gs{si}",
                                name=f"gs{si}_r{rep}")
                nc.sync.dma_start(
                    gt[:].rearrange("p (c f) -> p c f", c=3),
                    _slab(g_d.ap()[3 * si:3 * si + 3]),
                )
                gs.append(gt)

            # ---- kernel channels: prefill + add-cast x DMA, sigmoid, b/a ----
            for si, slab in enumerate(X_SLABS):
                n = len(slab)
                xs = xpool.tile([P, n * F], _f16, tag=f"xs{si}",
                                name=f"xs{si}_r{rep}")
                for j in range(n):
                    nc.vector.tensor_scalar(
                        xs[:, j * F:(j + 1) * F], M_t[:], BIG, -BIG,
                        _ALU.mult, _ALU.add,
                    )
                for j, k in enumerate(slab):
                    add_dma(xs[:, j * F:(j + 1) * F], k)
                for j, k in enumerate(slab):
                    sig = spool.tile([P, F], _f16, tag="sig", name=f"sig{k}_r{rep}")
                    nc.scalar.activation(
                        sig[:], xs[:, j * F:(j + 1) * F], _AF.Sigmoid
                    )
                    gsl = gs[k // 3]
                    if k in A_DVE:
                        pe_reduce(sig, sig, B_COL[k], f"b{k}")
                        ja = jpool.tile([P, F], _f16, tag="junk",
                                        name=f"ja{k}_r{rep}")
                        nc.vector.scalar_tensor_tensor(
                            ja[:], gsl[:, (k % 3) * F:(k % 3 + 1) * F], 1.0,
                            sig[:], _ALU.mult, _ALU.mult,
                            accum_out=acc_dve[:, A_COL[k]:A_COL[k] + 1],
                        )
                    else:
                        pe_reduce_ba(sig, gsl, k, k % 3)

            # ---- c sweep: all six c_k share M as stationary ----
            for si in range(2):
                cps = cpool_ps.tile([P, 3 * P], _f32, tag=f"cps{si}",
                                    name=f"cps{si}_r{rep}")
                g3 = gs[si][:].rearrange("p (c f) -> p c f", c=3)
                for t in range(NCH):
                    nc.tensor.matmul(
                        cps[:], lhsT=M_t[:, t * P:(t + 1) * P],
                        rhs=g3[:, :, t * P:(t + 1) * P],
                        start=(t == 0), stop=(t == NCH - 1),
                    )
                for j in range(3):
                    extract(cps[:, j * P:(j + 1) * P], C_COL[3 * si + j],
                            f"c{3 * si + j}")

        for rep in range(reps):
            image_body(rep)

        # cross-partition reduction of the accumulators (ones-matmul), then out
        pr = pfpool.tile([24, 1], _f32, tag="pr")
        nc.tensor.matmul(pr[:], lhsT=acc_dve[:], rhs=ones32[:],
                         start=True, stop=True)
        pr2 = pfpool.tile([2, 1], _f32, tag="pr2")
        nc.tensor.matmul(pr2[:], lhsT=acc_act[:], rhs=ones32[:],
                         start=True, stop=True)
        res_sb = cpool.tile([24, 1], _f32)
        nc.scalar.copy(res_sb[:], pr[:])
        res_sb2 = cpool.tile([2, 1], _f32)
        nc.scalar.copy(res_sb2[:], pr2[:])
        nc.sync.dma_start(res_d.ap()[0:24], res_sb[:])
        nc.sync.dma_start(res_d.ap()[24:26], res_sb2[:])

    nc.compile()
    return nc


_CACHE = {}


def _get_nc():
    if "nc" not in _CACHE:
        _CACHE["nc"] = build_nc(debug=False)
    return _CACHE["nc"]


def _combine(res_list):
    """res_list: per-image [26, 1] device sums -> (loss_text, loss_kernels, loss).

    Returns None if the OHEM fast-path precondition fails for any image.
    """
    lt_b = np.zeros(B, np.float64)
    lk_b = np.zeros(B, np.float64)
    for b in range(B):
        v = np.asarray(res_list[b], np.float64).reshape(-1)
        a_t = v[A_COL[6]]
        b_t = v[24]                      # acc_act col 0 -> res row 24
        c_t = v[C_COL[6]]
        pos_num = c_t                    # sum(gt_text * m), exact integer
        # sel == m iff pos_num == 0 (fallback) or RATIO*pos_num >= total_neg.
        # Since sum_g >= pos_num, (RATIO+1)*pos_num >= N is sufficient.
        if not (pos_num == 0 or (RATIO + 1) * pos_num >= N):
            return None
        lt_b[b] = 1.0 - 2.0 * a_t / (b_t + 0.001 + c_t + 0.001)
        lk = 0.0
        for k in range(NK):
            a_k = v[A_COL[k]]
            b_k = v[B_COL[k]]
            c_k = v[C_COL[k]]
            lk += 1.0 - 2.0 * a_k / (b_k + 0.001 + c_k + 0.001)
        lk_b[b] = lk / NK
    lt = np.float32(lt_b.mean())
    lk = np.float32(lk_b.mean())
    loss = np.float32(LAMBDA) * lt + np.float32(1.0 - LAMBDA) * lk
    return (lt, lk, np.float32(loss))


def _numpy_reference(outputs, labels, training_masks):
    """Full-fidelity host fallback (mirrors the original loss exactly)."""
    def sigmoid(z):
        return 1.0 / (1.0 + np.exp(-z, dtype=np.float64))

    texts = outputs[:, -1].reshape(B, N).astype(np.float64)
    kernels = outputs[:, :-1].reshape(B, NK, N).astype(np.float64)
    gt_texts = labels[:, -1].reshape(B, N).astype(np.float64)
    gt_kernels = labels[:, :-1].reshape(B, NK, N).astype(np.float64)
    tm = training_masks.reshape(B, N).astype(np.float64)

    pos = gt_texts > 0.5
    pos_num = np.sum(pos & (tm > 0.5), axis=1)
    neg = ~pos
    total_neg = np.sum(neg, axis=1)
    neg_num = np.minimum(pos_num * RATIO, total_neg)
    neg_scores = np.where(neg, texts, -np.inf)
    sorted_desc = -np.sort(-neg_scores, axis=1)
    idx = np.clip(neg_num - 1, 0, N - 1)
    thr = np.take_along_axis(sorted_desc, idx[:, None], axis=1)
    sel = (((texts >= thr) | pos) & (tm > 0.5)).astype(np.float64)
    fallback = (pos_num == 0) | (neg_num == 0)
    sel = np.where(fallback[:, None], tm, sel)

    def dice(inp, target, mask):
        p = sigmoid(inp) * mask
        t = target * mask
        a = np.sum(p * t, axis=-1)
        bb = np.sum(p * p, axis=-1) + 0.001
        cc = np.sum(t * t, axis=-1) + 0.001
        return 1.0 - 2.0 * a / (bb + cc)

    loss_text = dice(texts, gt_texts, sel).mean()
    sel_k = ((sigmoid(texts) > 0.5) & (tm > 0.5)).astype(np.float64)
    loss_kernels = dice(kernels, gt_kernels, sel_k[:, None, :]).mean(axis=1).mean()
    loss = LAMBDA * loss_text + (1.0 - LAMBDA) * loss_kernels
    return (np.float32(loss_text), np.float32(loss_kernels), np.float32(loss))


def kernel(outputs, labels, training_masks):
    outputs = np.asarray(outputs, dtype=np.float32)
    labels = np.asarray(labels, dtype=np.float32)
    training_masks = np.asarray(training_masks, dtype=np.float32)
    assert outputs.shape == (B, C, H, W)

    x8 = outputs.astype(ml_dtypes.float8_e3m4)
    g8 = labels.astype(ml_dtypes.float8_e4m3)
    m16 = training_masks.astype(np.float16)

    nc = _get_nc()
    in_maps = [
        {
            "x": np.ascontiguousarray(x8[b]),
            "g": np.ascontiguousarray(g8[b]),
            "m": np.ascontiguousarray(m16[b]),
        }
        for b in range(B)
    ]
    r = None
    for attempt in range(3):
        try:
            r = run_bass_kernel_spmd(
                nc, in_maps, list(range(NCORES)),
                trace=_CACHE.get("trace", False),
            )
            break
        except Exception:
            if attempt == 2:
                raise
            _CACHE.pop("nc", None)
            nc = _get_nc()
    _CACHE["last_result"] = r
    res_list = [r.results[b]["res"] for b in range(B)]
    out = _combine(res_list)
    if out is None:
        # OHEM threshold is not the minimum negative score -> exact host path
        out = _numpy_reference(outputs, labels, training_masks)
    return out


# revision 13
# speedup vs baseline: 16.5297x; 16.5297x over previous
"""PSENet-style OHEM + dice loss on 8 Trainium2 NeuronCores.

Data-parallel over the batch: core b processes image b entirely on-chip.
HBM traffic is minimized by staging inputs at low precision (tolerance is
rel-err < 2e-2): logits x as fp8 e3m4, labels g as fp8 e4m3 (0/1 exact),
training mask m as fp16 -- 6.6 MB/image instead of 24.6 MB fp32.

Per-channel masked dice sums (labels g, masks m/M are exactly 0/1):
  sig_k := sigmoid(x_k + (M-1)*BIG) = sigmoid(x_k)*M  (up to sig(-44) ~ 1e-19)
  a_k = sum(sig_k * g_k),  b_k = sum(sig_k^2),  c_k = sum(g_k * M)
The (M-1)*BIG mask-offset is applied *during the DMA*: the destination tile
is prefilled with (M-1)*BIG by a 4x-rate DVE tensor_scalar, and the x DMA
uses accum_op=add with an fp8->fp16 cast (SWDGE CCE). The text-channel mask
M = (x6>0)*m falls out as (xm6>0) of the already-offset text logits.

Dot-product reductions run mostly on the otherwise-idle TensorEngine via
chunked [128,128] accumulating matmuls whose PSUM diagonal holds the
answer (extracted by one DVE I-mask STT+accum per reduction):
  - all six c_k share M as the stationary operand and stream 3 g channels
    per matmul ([128,384] PSUM tiles, g consumed directly as fp8)
  - b_k/a_k share sig_k as the stationary operand
DVE reduce-class ops (accum_out) run at 1x regardless of dtype, so only 3
a_k reductions live on DVE; ACT does sigmoids + one square-accum.

Engine occupancy (cost-model, per image): PE ~23us, DVE ~22us, ACT ~23us,
Pool (SWDGE issue) ~17us, DMA queues ~17us.  Final scalars: accumulator
columns reduced across partitions by one ones-matmul; host combines
8 x 26 floats (OHEM fast path verified on host: sel == training_mask iff
(RATIO+1)*pos_num >= N, with ~28 sigma of margin; exact host fallback
otherwise).
"""

import os
import sys

import numpy as np
import ml_dtypes

for _p in ("/opt/trn_rl_repo", "/root/.axon_site/_ro/trn_rl_repo"):
    if os.path.isdir(_p) and _p not in sys.path:
        sys.path.append(_p)

import concourse.bacc as bacc
import concourse.tile as tile
from concourse import mybir
from concourse.bass_utils import run_bass_kernel_spmd

B, C, H, W = 8, 7, 640, 640
NK = C - 1            # kernel channels
N = H * W             # pixels per image
P = 128               # SBUF partitions
F = N // P            # free dim per plane tile (3200)
NCH = F // P          # 25 matmul chunks per plane reduction
BIG = 50.0
NCORES = 8
LAMBDA = 0.7
RATIO = 3

_f32 = mybir.dt.float32
_f16 = mybir.dt.float16
_f8x = mybir.dt.float8e3   # logits (e3m4: 4-bit mantissa, |x| < 15.5)
_f8g = mybir.dt.float8e4   # labels (0/1 exact)
_AF = mybir.ActivationFunctionType
_ALU = mybir.AluOpType

# accumulator column map (acc_dve [128, 24]; b6 -> acc_act col 0)
A_COL = {k: k for k in range(7)}
B_COL = {0: 7, 1: 8, 2: 9, 3: 10, 4: 11, 5: 12}
C_COL = {k: 14 + k for k in range(7)}
A_DVE = (0, 1, 6)          # a_k on DVE STT+accum; a2..a5 paired with b on PE
X_SLABS = [[0], [1, 2], [3, 4, 5]]   # kernel-channel x DMA grouping


def _plane(dram_ap):
    """[H, W] dram slab -> [128, 3200] partition-major access pattern."""
    return dram_ap.rearrange("(p q) w -> p (q w)", p=P)


def _slab(dram_ap):
    """[n, H, W] dram slab -> [128, n, 3200] (channel-major free dims)."""
    return dram_ap.rearrange("c (p q) w -> p c (q w)", p=P)


def build_nc(debug=False, reps=1):
    nc = bacc.Bacc("TRN2", target_bir_lowering=False, debug=debug)
    x_d = nc.dram_tensor("x", [C, H, W], _f8x, kind="ExternalInput")
    g_d = nc.dram_tensor("g", [C, H, W], _f8g, kind="ExternalInput")
    m_d = nc.dram_tensor("m", [H, W], _f8g, kind="ExternalInput")
    res_d = nc.dram_tensor("res", [26, 1], _f32, kind="ExternalOutput")

    with (
        tile.TileContext(nc) as tc,
        tc.tile_pool(name="const", bufs=1) as cpool,
        tc.tile_pool(name="xin", bufs=2) as xpool,
        tc.tile_pool(name="gin", bufs=2) as gpool,
        tc.tile_pool(name="sigp", bufs=3) as spool,
        tc.tile_pool(name="mp", bufs=2) as mpool,
        tc.tile_pool(name="junk", bufs=2) as jpool,
        tc.tile_pool(name="j128", bufs=2) as j128pool,
        tc.tile_pool(name="psba", bufs=2, space="PSUM") as bapool,
        tc.tile_pool(name="psc", bufs=1, space="PSUM") as cpool_ps,
        tc.tile_pool(name="psf", bufs=1, space="PSUM") as pfpool,
    ):
        ones32 = cpool.tile([P, 1], _f32)
        nc.gpsimd.memset(ones32[:], 1.0)
        iota_t = cpool.tile([P, P], mybir.dt.int32)
        nc.gpsimd.iota(iota_t[:], pattern=[[1, P]], base=0, channel_multiplier=-1)
        I_t = cpool.tile([P, P], _f16)
        nc.vector.tensor_scalar(I_t[:], iota_t[:], 0.0, None, _ALU.is_equal)
        acc_dve = cpool.tile([P, 24], _f32)
        acc_act = cpool.tile([P, 2], _f32)
        nc.vector.memset(acc_dve[:], 0.0)
        nc.scalar.memzero(acc_act[:])

        def image_body(rep):
            def extract(ps_slice, col, tag):
                jk = j128pool.tile([P, P], _f16, tag="j128", name=f"x{tag}_r{rep}")
                nc.vector.scalar_tensor_tensor(
                    jk[:], ps_slice, 1.0, I_t[:], _ALU.mult, _ALU.mult,
                    accum_out=acc_dve[:, col:col + 1],
                )

            def pe_reduce(u, v, col, tag):
                ps = bapool.tile([P, P], _f32, tag="bps", name=f"{tag}_r{rep}")
                for t in range(NCH):
                    nc.tensor.matmul(
                        ps[:], lhsT=u[:, t * P:(t + 1) * P],
                        rhs=v[:, t * P:(t + 1) * P],
                        start=(t == 0), stop=(t == NCH - 1),
                    )
                extract(ps[:], col, tag)

            def pe_reduce_ba(sig, gsl, k, j):
                # b_k = <sig,sig>, a_k = <sig,g>; sig is the shared stationary
                bps = bapool.tile([P, P], _f32, tag="bps", name=f"b{k}_r{rep}")
                aps = bapool.tile([P, P], _f32, tag="aps", name=f"a{k}_r{rep}")
                for t in range(NCH):
                    sl = sig[:, t * P:(t + 1) * P]
                    nc.tensor.matmul(bps[:], lhsT=sl, rhs=sl,
                                     start=(t == 0), stop=(t == NCH - 1))
                    nc.tensor.matmul(
                        aps[:], lhsT=sl,
                        rhs=gsl[:, j * F + t * P:j * F + (t + 1) * P],
                        start=(t == 0), stop=(t == NCH - 1),
                    )
                extract(bps[:], B_COL[k], f"b{k}")
                extract(aps[:], A_COL[k], f"a{k}")

            # ---- text channel ----
            def add_dma(dst, k):
                # CCE accumulate corrupts beyond 2048 elems/partition-row:
                # split each plane into two 1600-column halves.
                src = _plane(x_d.ap()[k])
                Fh = F // 2
                for h in range(2):
                    nc.gpsimd.dma_start(
                        dst[:, h * Fh:(h + 1) * Fh],
                        src[:, h * Fh:(h + 1) * Fh],
                        accum_op=_ALU.add,
                    )

            m_t = mpool.tile([P, F], _f16, tag="m", name=f"m_r{rep}")
            nc.gpsimd.dma_start(m_t[:], _plane(m_d.ap()))
            xm6 = xpool.tile([P, F], _f16, tag="x6", name=f"x6_r{rep}")
            nc.vector.tensor_scalar(
                xm6[:], m_t[:], BIG, -BIG, _ALU.mult, _ALU.add
            )
            add_dma(xm6, C - 1)
            g6 = gpool.tile([P, F], _f8g, tag="g6", name=f"g6_r{rep}")
            nc.sync.dma_start(g6[:], _plane(g_d.ap()[C - 1]))

            sig6 = spool.tile([P, F], _f16, tag="sig", name=f"sig6_r{rep}")
            nc.scalar.activation(sig6[:], xm6[:], _AF.Sigmoid)
            M_t = mpool.tile([P, F], _f16, tag="M", name=f"M_r{rep}")
            nc.vector.tensor_scalar(M_t[:], xm6[:], 0.0, None, _ALU.is_gt)

            # b6 on ACT; a6 on DVE; c6 = <g6, m> on PE
            jb6 = jpool.tile([P, F], _f16, tag="junk", name=f"jb6_r{rep}")
            nc.scalar.activation(jb6[:], sig6[:], _AF.Square,
                                 accum_out=acc_act[:, 0:1])
            ja6 = jpool.tile([P, F], _f16, tag="junk", name=f"ja6_r{rep}")
            nc.vector.scalar_tensor_tensor(
                ja6[:], g6[:], 1.0, sig6[:], _ALU.mult, _ALU.mult,
                accum_out=acc_dve[:, A_COL[6]:A_COL[6] + 1],
            )
            pe_reduce(g6, m_t, C_COL[6], "c6")

            # ---- kernel-channel g slabs (raw fp8, HWDGE) ----
            gs = []
            for si in range(2):
                gt = gpool.tile([P, 3 * F], _f8g, tag=f"gs{si}",
                                name=f"gs{si}_r{rep}")
                nc.sync.dma_start(
                    gt[:].rearrange("p (c f) -> p c f", c=3),
                    _slab(g_d.ap()[3 * si:3 * si + 3]),
                )
                gs.append(gt)

            # ---- kernel channels: prefill + add-cast x DMA, sigmoid, b/a ----
            for si, slab in enumerate(X_SLABS):
                n = len(slab)
                xs = xpool.tile([P, n * F], _f16, tag=f"xs{si}",
                                name=f"xs{si}_r{rep}")
                for j in range(n):
                    nc.vector.tensor_scalar(
                        xs[:, j * F:(j + 1) * F], M_t[:], BIG, -BIG,
                        _ALU.mult, _ALU.add,
                    )
                for j, k in enumerate(slab):
                    add_dma(xs[:, j * F:(j + 1) * F], k)
                for j, k in enumerate(slab):
                    sig = spool.tile([P, F], _f16, tag="sig", name=f"sig{k}_r{rep}")
                    nc.scalar.activation(
                        sig[:], xs[:, j * F:(j + 1) * F], _AF.Sigmoid
                    )
                    gsl = gs[k // 3]
                    if k in A_DVE:
                        pe_reduce(sig, sig, B_COL[k], f"b{k}")
                        ja = jpool.tile([P, F], _f16, tag="junk",
                                        name=f"ja{k}_r{rep}")
                        nc.vector.scalar_tensor_tensor(
                            ja[:], gsl[:, (k % 3) * F:(k % 3 + 1) * F], 1.0,
                            sig[:], _ALU.mult, _ALU.mult,
                            accum_out=acc_dve[:, A_COL[k]:A_COL[k] + 1],
                        )
                    else:
                        pe_reduce_ba(sig, gsl, k, k % 3)

            # ---- c sweep: all six c_k share M as stationary ----
            for si in range(2):
                cps = cpool_ps.tile([P, 3 * P], _f32, tag=f"cps{si}",
                                    name=f"cps{si}_r{rep}")
                g3 = gs[si][:].rearrange("p (c f) -> p c f", c=3)
                for t in range(NCH):
                    nc.tensor.matmul(
                        cps[:], lhsT=M_t[:, t * P:(t + 1) * P],
                        rhs=g3[:, :, t * P:(t + 1) * P],
                        start=(t == 0), stop=(t == NCH - 1),
                    )
                for j in range(3):
                    extract(cps[:, j * P:(j + 1) * P], C_COL[3 * si + j],
                            f"c{3 * si + j}")

        for rep in range(reps):
            image_body(rep)

        # cross-partition reduction of the accumulators (ones-matmul), then out
        pr = pfpool.tile([24, 1], _f32, tag="pr")
        nc.tensor.matmul(pr[:], lhsT=acc_dve[:], rhs=ones32[:],
                         start=True, stop=True)
        pr2 = pfpool.tile([2, 1], _f32, tag="pr2")
        nc.tensor.matmul(pr2[:], lhsT=acc_act[:], rhs=ones32[:],
                         start=True, stop=True)
        res_sb = cpool.tile([24, 1], _f32)
        nc.scalar.copy(res_sb[:], pr[:])
        res_sb2 = cpool.tile([2, 1], _f32)
        nc.scalar.copy(res_sb2[:], pr2[:])
        nc.sync.dma_start(res_d.ap()[0:24], res_sb[:])
        nc.sync.dma_start(res_d.ap()[24:26], res_sb2[:])

    nc.compile()
    return nc


_CACHE = {}


def _get_nc():
    if "nc" not in _CACHE:
        _CACHE["nc"] = build_nc(debug=False)
    return _CACHE["nc"]


def _combine(res_list):
    """res_list: per-image [26, 1] device sums -> (loss_text, loss_kernels, loss).

    Returns None if the OHEM fast-path precondition fails for any image.
    """
    lt_b = np.zeros(B, np.float64)
    lk_b = np.zeros(B, np.float64)
    for b in range(B):
        v = np.asarray(res_list[b], np.float64).reshape(-1)
        a_t = v[A_COL[6]]
        b_t = v[24]                      # acc_act col 0 -> res row 24
        c_t = v[C_COL[6]]
        pos_num = c_t                    # sum(gt_text * m), exact integer
        # sel == m iff pos_num == 0 (fallback) or RATIO*pos_num >= total_neg.
        # Since sum_g >= pos_num, (RATIO+1)*pos_num >= N is sufficient.
        if not (pos_num == 0 or (RATIO + 1) * pos_num >= N):
            return None
        lt_b[b] = 1.0 - 2.0 * a_t / (b_t + 0.001 + c_t + 0.001)
        lk = 0.0
        for k in range(NK):
            a_k = v[A_COL[k]]
            b_k = v[B_COL[k]]
            c_k = v[C_COL[k]]
            lk += 1.0 - 2.0 * a_k / (b_k + 0.001 + c_k + 0.001)
        lk_b[b] = lk / NK
    lt = np.float32(lt_b.mean())
    lk = np.float32(lk_b.mean())
    loss = np.float32(LAMBDA) * lt + np.float32(1.0 - LAMBDA) * lk
    return (lt, lk, np.float32(loss))


def _numpy_reference(outputs, labels, training_masks):
    """Full-fidelity host fallback (mirrors the original loss exactly)."""
    def sigmoid(z):
        return 1.0 / (1.0 + np.exp(-z, dtype=np.float64))

    texts = outputs[:, -1].reshape(B, N).astype(np.float64)
    kernels = outputs[:, :-1].reshape(B, NK, N).astype(np.float64)
    gt_texts = labels[:, -1].reshape(B, N).astype(np.float64)
    gt_kernels = labels[:, :-1].reshape(B, NK, N).astype(np.float64)
    tm = training_masks.reshape(B, N).astype(np.float64)

    pos = gt_texts > 0.5
    pos_num = np.sum(pos & (tm > 0.5), axis=1)
    neg = ~pos
    total_neg = np.sum(neg, axis=1)
    neg_num = np.minimum(pos_num * RATIO, total_neg)
    neg_scores = np.where(neg, texts, -np.inf)
    sorted_desc = -np.sort(-neg_scores, axis=1)
    idx = np.clip(neg_num - 1, 0, N - 1)
    thr = np.take_along_axis(sorted_desc, idx[:, None], axis=1)
    sel = (((texts >= thr) | pos) & (tm > 0.5)).astype(np.float64)
    fallback = (pos_num == 0) | (neg_num == 0)
    sel = np.where(fallback[:, None], tm, sel)

    def dice(inp, target, mask):
        p = sigmoid(inp) * mask
        t = target * mask
        a = np.sum(p * t, axis=-1)
        bb = np.sum(p * p, axis=-1) + 0.001
        cc = np.sum(t * t, axis=-1) + 0.001
        return 1.0 - 2.0 * a / (bb + cc)

    loss_text = dice(texts, gt_texts, sel).mean()
    sel_k = ((sigmoid(texts) > 0.5) & (tm > 0.5)).astype(np.float64)
    loss_kernels = dice(kernels, gt_kernels, sel_k[:, None, :]).mean(axis=1).mean()
    loss = LAMBDA * loss_text + (1.0 - LAMBDA) * loss_kernels
    return (np.float32(loss_text), np.float32(loss_kernels), np.float32(loss))


def kernel(outputs, labels, training_masks):
    outputs = np.asarray(outputs, dtype=np.float32)
    labels = np.asarray(labels, dtype=np.float32)
    training_masks = np.asarray(training_masks, dtype=np.float32)
    assert outputs.shape == (B, C, H, W)

    x8 = outputs.astype(ml_dtypes.float8_e3m4)
    g8 = labels.astype(ml_dtypes.float8_e4m3)
    m8 = training_masks.astype(ml_dtypes.float8_e4m3)

    nc = _get_nc()
    in_maps = [
        {
            "x": np.ascontiguousarray(x8[b]),
            "g": np.ascontiguousarray(g8[b]),
            "m": np.ascontiguousarray(m8[b]),
        }
        for b in range(B)
    ]
    r = None
    for attempt in range(3):
        try:
            r = run_bass_kernel_spmd(
                nc, in_maps, list(range(NCORES)),
                trace=_CACHE.get("trace", False),
            )
            break
        except Exception:
            if attempt == 2:
                raise
            _CACHE.pop("nc", None)
            nc = _get_nc()
    _CACHE["last_result"] = r
    res_list = [r.results[b]["res"] for b in range(B)]
    out = _combine(res_list)
    if out is None:
        # OHEM threshold is not the minimum negative score -> exact host path
        out = _numpy_reference(outputs, labels, training_masks)
    return out


# revision 14
# speedup vs baseline: 17.1196x; 1.0357x over previous
"""PSENet-style OHEM + dice loss on 8 Trainium2 NeuronCores.

Data-parallel over the batch: core b processes image b entirely on-chip.
HBM traffic is minimized by staging inputs at low precision (tolerance is
rel-err < 2e-2; measured 2.6e-5): logits x as fp8 e3m4, labels g and mask m
as fp8 e4m3 (0/1 exact) -- 6.15 MB/image instead of 24.6 MB fp32, which
puts the kernel at the HBM roofline (~17.5 us/image measured vs 72.4 us
for the fp32 baseline).

Per-channel masked dice sums (labels g, masks m/M are exactly 0/1):
  sig_k := sigmoid(x_k + (M-1)*BIG) = sigmoid(x_k)*M  (up to sig(-44) ~ 1e-19)
  a_k = sum(sig_k * g_k),  b_k = sum(sig_k^2),  c_k = sum(g_k * M)
The (M-1)*BIG mask-offset is applied *during the DMA*: the destination tile
is prefilled with (M-1)*BIG by a 4x-rate DVE tensor_scalar, and the x DMA
uses accum_op=add with an fp8->fp16 cast (SWDGE CCE). The text-channel mask
M = (x6>0)*m falls out as (xm6>0) of the already-offset text logits.

Dot-product reductions run mostly on the otherwise-idle TensorEngine via
chunked [128,128] accumulating matmuls whose PSUM diagonal holds the
answer (extracted by one DVE I-mask STT+accum per reduction):
  - all six c_k share M as the stationary operand and stream 3 g channels
    per matmul ([128,384] PSUM tiles, g consumed directly as fp8)
  - b_k/a_k share sig_k as the stationary operand
DVE reduce-class ops (accum_out) run at 1x regardless of dtype, so only 3
a_k reductions live on DVE; ACT does sigmoids + one square-accum.

Engine occupancy (cost-model, per image): PE ~23us, DVE ~22us, ACT ~23us,
Pool (SWDGE issue) ~17us, DMA queues ~17us.  Final scalars: accumulator
columns reduced across partitions by one ones-matmul; host combines
8 x 26 floats (OHEM fast path verified on host: sel == training_mask iff
(RATIO+1)*pos_num >= N, with ~28 sigma of margin; exact host fallback
otherwise).
"""

import os
import sys

import numpy as np
import ml_dtypes

for _p in ("/opt/trn_rl_repo", "/root/.axon_site/_ro/trn_rl_repo"):
    if os.path.isdir(_p) and _p not in sys.path:
        sys.path.append(_p)

import concourse.bacc as bacc
import concourse.tile as tile
from concourse import mybir
from concourse.bass_utils import run_bass_kernel_spmd

B, C, H, W = 8, 7, 640, 640
NK = C - 1            # kernel channels
N = H * W             # pixels per image
P = 128               # SBUF partitions
F = N // P            # free dim per plane tile (3200)
NCH = F // P          # 25 matmul chunks per plane reduction
BIG = 50.0
NCORES = 8
LAMBDA = 0.7
RATIO = 3

_f32 = mybir.dt.float32
_f16 = mybir.dt.float16
_f8x = mybir.dt.float8e3   # logits (e3m4: 4-bit mantissa, |x| < 15.5)
_f8g = mybir.dt.float8e4   # labels (0/1 exact)
_AF = mybir.ActivationFunctionType
_ALU = mybir.AluOpType

# accumulator column map (acc_dve [128, 24]; b6 -> acc_act col 0)
A_COL = {k: k for k in range(7)}
B_COL = {0: 7, 1: 8, 2: 9, 3: 10, 4: 11, 5: 12}
C_COL = {k: 14 + k for k in range(7)}
A_DVE = (0, 1, 6)          # a_k on DVE STT+accum; a2..a5 paired with b on PE
X_SLABS = [[0], [1, 2], [3, 4, 5]]   # kernel-channel x DMA grouping


def _plane(dram_ap):
    """[H, W] dram slab -> [128, 3200] partition-major access pattern."""
    return dram_ap.rearrange("(p q) w -> p (q w)", p=P)


def _slab(dram_ap):
    """[n, H, W] dram slab -> [128, n, 3200] (channel-major free dims)."""
    return dram_ap.rearrange("c (p q) w -> p c (q w)", p=P)


def build_nc(debug=False, reps=1):
    nc = bacc.Bacc("TRN2", target_bir_lowering=False, debug=debug)
    x_d = nc.dram_tensor("x", [C, H, W], _f8x, kind="ExternalInput")
    g_d = nc.dram_tensor("g", [C, H, W], _f8g, kind="ExternalInput")
    m_d = nc.dram_tensor("m", [H, W], _f8g, kind="ExternalInput")
    res_d = nc.dram_tensor("res", [26, 1], _f32, kind="ExternalOutput")

    with (
        tile.TileContext(nc) as tc,
        tc.tile_pool(name="const", bufs=1) as cpool,
        tc.tile_pool(name="xin", bufs=2) as xpool,
        tc.tile_pool(name="gin", bufs=2) as gpool,
        tc.tile_pool(name="sigp", bufs=3) as spool,
        tc.tile_pool(name="mp", bufs=2) as mpool,
        tc.tile_pool(name="junk", bufs=2) as jpool,
        tc.tile_pool(name="j128", bufs=2) as j128pool,
        tc.tile_pool(name="psba", bufs=2, space="PSUM") as bapool,
        tc.tile_pool(name="psc", bufs=1, space="PSUM") as cpool_ps,
        tc.tile_pool(name="psf", bufs=1, space="PSUM") as pfpool,
    ):
        ones32 = cpool.tile([P, 1], _f32)
        nc.gpsimd.memset(ones32[:], 1.0)
        iota_t = cpool.tile([P, P], mybir.dt.int32)
        nc.gpsimd.iota(iota_t[:], pattern=[[1, P]], base=0, channel_multiplier=-1)
        I_t = cpool.tile([P, P], _f16)
        nc.vector.tensor_scalar(I_t[:], iota_t[:], 0.0, None, _ALU.is_equal)
        acc_dve = cpool.tile([P, 24], _f32)
        acc_act = cpool.tile([P, 2], _f32)
        nc.vector.memset(acc_dve[:], 0.0)
        nc.scalar.memzero(acc_act[:])

        def image_body(rep):
            def extract(ps_slice, col, tag):
                jk = j128pool.tile([P, P], _f16, tag="j128", name=f"x{tag}_r{rep}")
                nc.vector.scalar_tensor_tensor(
                    jk[:], ps_slice, 1.0, I_t[:], _ALU.mult, _ALU.mult,
                    accum_out=acc_dve[:, col:col + 1],
                )

            def pe_reduce(u, v, col, tag):
                ps = bapool.tile([P, P], _f32, tag="bps", name=f"{tag}_r{rep}")
                for t in range(NCH):
                    nc.tensor.matmul(
                        ps[:], lhsT=u[:, t * P:(t + 1) * P],
                        rhs=v[:, t * P:(t + 1) * P],
                        start=(t == 0), stop=(t == NCH - 1),
                    )
                extract(ps[:], col, tag)

            def pe_reduce_ba(sig, gsl, k, j):
                # b_k = <sig,sig>, a_k = <sig,g>; sig is the shared stationary
                bps = bapool.tile([P, P], _f32, tag="bps", name=f"b{k}_r{rep}")
                aps = bapool.tile([P, P], _f32, tag="aps", name=f"a{k}_r{rep}")
                for t in range(NCH):
                    sl = sig[:, t * P:(t + 1) * P]
                    nc.tensor.matmul(bps[:], lhsT=sl, rhs=sl,
                                     start=(t == 0), stop=(t == NCH - 1))
                    nc.tensor.matmul(
                        aps[:], lhsT=sl,
                        rhs=gsl[:, j * F + t * P:j * F + (t + 1) * P],
                        start=(t == 0), stop=(t == NCH - 1),
                    )
                extract(bps[:], B_COL[k], f"b{k}")
                extract(aps[:], A_COL[k], f"a{k}")

            # ---- text channel ----
            def add_dma(dst, k):
                # CCE accumulate corrupts beyond 2048 elems/partition-row:
                # split each plane into two 1600-column halves.
                src = _plane(x_d.ap()[k])
                Fh = F // 2
                for h in range(2):
                    nc.gpsimd.dma_start(
                        dst[:, h * Fh:(h + 1) * Fh],
                        src[:, h * Fh:(h + 1) * Fh],
                        accum_op=_ALU.add,
                    )

            m_t = mpool.tile([P, F], _f16, tag="m", name=f"m_r{rep}")
            nc.gpsimd.dma_start(m_t[:], _plane(m_d.ap()))
            xm6 = xpool.tile([P, F], _f16, tag="x6", name=f"x6_r{rep}")
            nc.vector.tensor_scalar(
                xm6[:], m_t[:], BIG, -BIG, _ALU.mult, _ALU.add
            )
            add_dma(xm6, C - 1)
            g6 = gpool.tile([P, F], _f8g, tag="g6", name=f"g6_r{rep}")
            nc.sync.dma_start(g6[:], _plane(g_d.ap()[C - 1]))

            sig6 = spool.tile([P, F], _f16, tag="sig", name=f"sig6_r{rep}")
            nc.scalar.activation(sig6[:], xm6[:], _AF.Sigmoid)
            M_t = mpool.tile([P, F], _f16, tag="M", name=f"M_r{rep}")
            nc.vector.tensor_scalar(M_t[:], xm6[:], 0.0, None, _ALU.is_gt)

            # b6 on ACT; a6 on DVE; c6 = <g6, m> on PE
            jb6 = jpool.tile([P, F], _f16, tag="junk", name=f"jb6_r{rep}")
            nc.scalar.activation(jb6[:], sig6[:], _AF.Square,
                                 accum_out=acc_act[:, 0:1])
            ja6 = jpool.tile([P, F], _f16, tag="junk", name=f"ja6_r{rep}")
            nc.vector.scalar_tensor_tensor(
                ja6[:], g6[:], 1.0, sig6[:], _ALU.mult, _ALU.mult,
                accum_out=acc_dve[:, A_COL[6]:A_COL[6] + 1],
            )
            pe_reduce(g6, m_t, C_COL[6], "c6")

            # ---- kernel-channel g slabs (raw fp8, HWDGE) ----
            gs = []
            for si in range(2):
                gt = gpool.tile([P, 3 * F], _f8g, tag=f"gs{si}",
                                name=f"gs{si}_r{rep}")
                nc.sync.dma_start(
                    gt[:].rearrange("p (c f) -> p c f", c=3),
                    _slab(g_d.ap()[3 * si:3 * si + 3]),
                )
                gs.append(gt)

            # ---- kernel channels: prefill + add-cast x DMA, sigmoid, b/a ----
            for si, slab in enumerate(X_SLABS):
                n = len(slab)
                xs = xpool.tile([P, n * F], _f16, tag=f"xs{si}",
                                name=f"xs{si}_r{rep}")
                for j in range(n):
                    nc.vector.tensor_scalar(
                        xs[:, j * F:(j + 1) * F], M_t[:], BIG, -BIG,
                        _ALU.mult, _ALU.add,
                    )
                for j, k in enumerate(slab):
                    add_dma(xs[:, j * F:(j + 1) * F], k)
                for j, k in enumerate(slab):
                    sig = spool.tile([P, F], _f16, tag="sig", name=f"sig{k}_r{rep}")
                    nc.scalar.activation(
                        sig[:], xs[:, j * F:(j + 1) * F], _AF.Sigmoid
                    )
                    gsl = gs[k // 3]
                    if k in A_DVE:
                        pe_reduce(sig, sig, B_COL[k], f"b{k}")
                        ja = jpool.tile([P, F], _f16, tag="junk",
                                        name=f"ja{k}_r{rep}")
                        nc.vector.scalar_tensor_tensor(
                            ja[:], gsl[:, (k % 3) * F:(k % 3 + 1) * F], 1.0,
                            sig[:], _ALU.mult, _ALU.mult,
                            accum_out=acc_dve[:, A_COL[k]:A_COL[k] + 1],
                        )
                    else:
                        pe_reduce_ba(sig, gsl, k, k % 3)

            # ---- c sweep: all six c_k share M as stationary ----
            for si in range(2):
                cps = cpool_ps.tile([P, 3 * P], _f32, tag=f"cps{si}",
                                    name=f"cps{si}_r{rep}")
                g3 = gs[si][:].rearrange("p (c f) -> p c f", c=3)
                for t in range(NCH):
                    nc.tensor.matmul(
                        cps[:], lhsT=M_t[:, t * P:(t + 1) * P],
                        rhs=g3[:, :, t * P:(t + 1) * P],
                        start=(t == 0), stop=(t == NCH - 1),
                    )
                for j in range(3):
                    extract(cps[:, j * P:(j + 1) * P], C_COL[3 * si + j],
                            f"c{3 * si + j}")

        for rep in range(reps):
            image_body(rep)

        # cross-partition reduction of the accumulators (ones-matmul), then out
        pr = pfpool.tile([24, 1], _f32, tag="pr")
        nc.tensor.matmul(pr[:], lhsT=acc_dve[:], rhs=ones32[:],
                         start=True, stop=True)
        pr2 = pfpool.tile([2, 1], _f32, tag="pr2")
        nc.tensor.matmul(pr2[:], lhsT=acc_act[:], rhs=ones32[:],
                         start=True, stop=True)
        res_sb = cpool.tile([24, 1], _f32)
        nc.scalar.copy(res_sb[:], pr[:])
        res_sb2 = cpool.tile([2, 1], _f32)
        nc.scalar.copy(res_sb2[:], pr2[:])
        nc.sync.dma_start(res_d.ap()[0:24], res_sb[:])
        nc.sync.dma_start(res_d.ap()[24:26], res_sb2[:])

    nc.compile()
    return nc


_CACHE = {}


def _get_nc():
    if "nc" not in _CACHE:
        _CACHE["nc"] = build_nc(debug=False)
    return _CACHE["nc"]


def _combine(res_list):
    """res_list: per-image [26, 1] device sums -> (loss_text, loss_kernels, loss).

    Returns None if the OHEM fast-path precondition fails for any image.
    """
    lt_b = np.zeros(B, np.float64)
    lk_b = np.zeros(B, np.float64)
    for b in range(B):
        v = np.asarray(res_list[b], np.float64).reshape(-1)
        a_t = v[A_COL[6]]
        b_t = v[24]                      # acc_act col 0 -> res row 24
        c_t = v[C_COL[6]]
        pos_num = c_t                    # sum(gt_text * m), exact integer
        # sel == m iff pos_num == 0 (fallback) or RATIO*pos_num >= total_neg.
        # Since sum_g >= pos_num, (RATIO+1)*pos_num >= N is sufficient.
        if not (pos_num == 0 or (RATIO + 1) * pos_num >= N):
            return None
        lt_b[b] = 1.0 - 2.0 * a_t / (b_t + 0.001 + c_t + 0.001)
        lk = 0.0
        for k in range(NK):
            a_k = v[A_COL[k]]
            b_k = v[B_COL[k]]
            c_k = v[C_COL[k]]
            lk += 1.0 - 2.0 * a_k / (b_k + 0.001 + c_k + 0.001)
        lk_b[b] = lk / NK
    lt = np.float32(lt_b.mean())
    lk = np.float32(lk_b.mean())
    loss = np.float32(LAMBDA) * lt + np.float32(1.0 - LAMBDA) * lk
    return (lt, lk, np.float32(loss))


def _numpy_reference(outputs, labels, training_masks):
    """Full-fidelity host fallback (mirrors the original loss exactly)."""
    def sigmoid(z):
        return 1.0 / (1.0 + np.exp(-z, dtype=np.float64))

    texts = outputs[:, -1].reshape(B, N).astype(np.float64)
    kernels = outputs[:, :-1].reshape(B, NK, N).astype(np.float64)
    gt_texts = labels[:, -1].reshape(B, N).astype(np.float64)
    gt_kernels = labels[:, :-1].reshape(B, NK, N).astype(np.float64)
    tm = training_masks.reshape(B, N).astype(np.float64)

    pos = gt_texts > 0.5
    pos_num = np.sum(pos & (tm > 0.5), axis=1)
    neg = ~pos
    total_neg = np.sum(neg, axis=1)
    neg_num = np.minimum(pos_num * RATIO, total_neg)
    neg_scores = np.where(neg, texts, -np.inf)
    sorted_desc = -np.sort(-neg_scores, axis=1)
    idx = np.clip(neg_num - 1, 0, N - 1)
    thr = np.take_along_axis(sorted_desc, idx[:, None], axis=1)
    sel = (((texts >= thr) | pos) & (tm > 0.5)).astype(np.float64)
    fallback = (pos_num == 0) | (neg_num == 0)
    sel = np.where(fallback[:, None], tm, sel)

    def dice(inp, target, mask):
        p = sigmoid(inp) * mask
        t = target * mask
        a = np.sum(p * t, axis=-1)
        bb = np.sum(p * p, axis=-1) + 0.001
        cc = np.sum(t * t, axis=-1) + 0.001
        return 1.0 - 2.0 * a / (bb + cc)

    loss_text = dice(texts, gt_texts, sel).mean()
    sel_k = ((sigmoid(texts) > 0.5) & (tm > 0.5)).astype(np.float64)
    loss_kernels = dice(kernels, gt_kernels, sel_k[:, None, :]).mean(axis=1).mean()
    loss = LAMBDA * loss_text + (1.0 - LAMBDA) * loss_kernels
    return (np.float32(loss_text), np.float32(loss_kernels), np.float32(loss))


def kernel(outputs, labels, training_masks):
    outputs = np.asarray(outputs, dtype=np.float32)
    labels = np.asarray(labels, dtype=np.float32)
    training_masks = np.asarray(training_masks, dtype=np.float32)
    assert outputs.shape == (B, C, H, W)

    x8 = outputs.astype(ml_dtypes.float8_e3m4)
    g8 = labels.astype(ml_dtypes.float8_e4m3)
    m8 = training_masks.astype(ml_dtypes.float8_e4m3)

    nc = _get_nc()
    in_maps = [
        {
            "x": np.ascontiguousarray(x8[b]),
            "g": np.ascontiguousarray(g8[b]),
            "m": np.ascontiguousarray(m8[b]),
        }
        for b in range(B)
    ]
    r = None
    for attempt in range(3):
        try:
            r = run_bass_kernel_spmd(
                nc, in_maps, list(range(NCORES)),
                trace=_CACHE.get("trace", False),
            )
            break
        except Exception:
            if attempt == 2:
                raise
            _CACHE.pop("nc", None)
            nc = _get_nc()
    _CACHE["last_result"] = r
    res_list = [r.results[b]["res"] for b in range(B)]
    out = _combine(res_list)
    if out is None:
        # OHEM threshold is not the minimum negative score -> exact host path
        out = _numpy_reference(outputs, labels, training_masks)
    return out
